# revision 1
# baseline (speedup 1.0000x reference)
"""MoE grouped-GEMM expert FFN (SwiGLU) on 8 Trainium2 NeuronCores.

Expert-parallel sharding: tokens arrive pre-grouped by expert with uniform
group size g = T/E = 1024, so core c owns experts [4c, 4c+4) and token rows
[c*4096, (c+1)*4096). No cross-core communication is needed: each core
computes its own 4 experts' FFN on its own token block.

Per-core math, per expert e:
    gu^T = w13_e^T-chunks @ x_e^T        # PE: contract H on partitions
    h^T  = silu(gate^T) * up^T           # ACT (Silu) + DVE (mul), bf16 out
    out  = h @ w2_e                      # PE: contract I on partitions

The host pre-transposes x (so H lands on SBUF partitions) and pre-tiles the
weights into [128, free] k-tiles, giving every DMA >=1KB contiguous
per-partition lines. All matmuls are 128x128 stationary x [128,512] moving,
bf16 in / fp32 PSUM accumulate.
"""

import sys

if "/opt/trn_rl_repo" not in sys.path:
    sys.path.insert(0, "/opt/trn_rl_repo")

import ml_dtypes
import numpy as np

import concourse.bacc as bacc
import concourse.bass as bass
import concourse.mybir as mybir
from concourse import tile
from concourse.bass_utils import run_bass_kernel_spmd

BF16 = mybir.dt.bfloat16
F32 = mybir.dt.float32
NPBF16 = ml_dtypes.bfloat16

N_CORES = 8
E = 32
H = 2048
I = 1024
T = 32768
EPC = E // N_CORES          # experts per core = 4
G = T // E                  # tokens per expert = 1024
ROWS = EPC * G              # token rows per core = 4096
KH = H // 128               # 16 contraction tiles for GEMM1
KI = I // 128               # 8 contraction tiles for GEMM2


def build_nc():
    nc = bacc.Bacc()
    xt_d = nc.declare_dram_parameter("xt", [KH, 128, ROWS], BF16, isOutput=False)
    w13_d = nc.declare_dram_parameter("w13", [EPC, KH, 128, 2 * I], BF16, isOutput=False)
    w2_d = nc.declare_dram_parameter("w2", [EPC, KI, 128, H], BF16, isOutput=False)
    out_d = nc.declare_dram_parameter("out", [ROWS, H], F32, isOutput=True)

    with tile.TileContext(nc) as tc:
        with (
            tc.tile_pool(name="xt", bufs=1) as xt_pool,
            tc.tile_pool(name="w13", bufs=1) as w13_pool,
            tc.tile_pool(name="w2", bufs=1) as w2_pool,
            tc.tile_pool(name="h", bufs=2) as h_pool,
            tc.tile_pool(name="tmp", bufs=3) as tmp_pool,
            tc.tile_pool(name="ost", bufs=4) as ost_pool,
            tc.tile_pool(name="ps", bufs=2, space="PSUM") as ps_pool,
        ):
            for e in range(EPC):
                xt_sb = []
                for k in range(KH):
                    t = xt_pool.tile([128, G], BF16, tag=f"xt{k}", bufs=1, name=f"xt{k}_{e}")
                    nc.sync.dma_start(t[:], xt_d[k][:, e * G:(e + 1) * G])
                    xt_sb.append(t)
                w13_sb = []
                for k in range(KH):
                    t = w13_pool.tile([128, 2 * I], BF16, tag=f"w13_{k}", bufs=1, name=f"w13_{k}_{e}")
                    nc.sync.dma_start(t[:], w13_d[e, k][:])
                    w13_sb.append(t)
                w2_sb = []
                for k in range(KI):
                    t = w2_pool.tile([128, H], BF16, tag=f"w2_{k}", bufs=1, name=f"w2_{k}_{e}")
                    nc.sync.dma_start(t[:], w2_d[e, k][:])
                    w2_sb.append(t)

                # Phase 1: gu^T tiles -> SwiGLU -> h^T resident in SBUF (bf16).
                h_sb = [h_pool.tile([128, G], BF16, tag=f"h{m}", bufs=2, name=f"h{m}_{e}") for m in range(KI)]
                for m in range(KI):
                    # One PSUM bank per (gate/up, n) group; the k-loop
                    # interleaves all four so each stationary weight tile
                    # feeds two consecutive matmuls (LDW reuse) and the PE
                    # pipelines fills across banks.
                    pg = [ps_pool.tile([128, 512], F32, tag=f"pg{n}", bufs=1, name=f"pg{n}_{e}_{m}")
                          for n in range(2)]
                    pu = [ps_pool.tile([128, 512], F32, tag=f"pu{n}", bufs=1, name=f"pu{n}_{e}_{m}")
                          for n in range(2)]
                    for k in range(KH):
                        wg = w13_sb[k][:, m * 128:(m + 1) * 128]
                        wu = w13_sb[k][:, I + m * 128:I + (m + 1) * 128]
                        for n in range(2):
                            nc.tensor.matmul(
                                pg[n][:], wg, xt_sb[k][:, n * 512:(n + 1) * 512],
                                start=(k == 0), stop=(k == KH - 1),
                            )
                        for n in range(2):
                            nc.tensor.matmul(
                                pu[n][:], wu, xt_sb[k][:, n * 512:(n + 1) * 512],
                                start=(k == 0), stop=(k == KH - 1),
                            )
                    for n in range(2):
                        ncol = slice(n * 512, (n + 1) * 512)
                        tmp = tmp_pool.tile([128, 512], F32, tag="tmp", bufs=3, name=f"tmp_{e}_{m}_{n}")
                        pu_sb = tmp_pool.tile([128, 512], F32, tag="pusb", bufs=3, name=f"pusb_{e}_{m}_{n}")
                        nc.scalar.activation(
                            tmp[:], pg[n][:], mybir.ActivationFunctionType.Silu
                        )
                        # Both epilogue producers run on ACT so the DVE mul
                        # carries ONE merged ACT wait (the TT instruction
                        # encoding only fits a single sync-wait).
                        nc.scalar.copy(pu_sb[:], pu[n][:])
                        nc.vector.tensor_mul(h_sb[m][:, ncol], tmp[:], pu_sb[:])

                # Phase 2: out_e = h @ w2_e, streamed straight to DRAM.
                for mt in range(KI):
                    rows = slice(e * G + mt * 128, e * G + (mt + 1) * 128)
                    po = [ps_pool.tile([128, 512], F32, tag=f"po{n}", bufs=1, name=f"po{n}_{e}_{mt}")
                          for n in range(4)]
                    for k in range(KI):
                        hk = h_sb[k][:, mt * 128:(mt + 1) * 128]
                        for n in range(4):
                            nc.tensor.matmul(
                                po[n][:], hk, w2_sb[k][:, n * 512:(n + 1) * 512],
                                start=(k == 0), stop=(k == KI - 1),
                            )
                    for n in range(4):
                        ncol = slice(n * 512, (n + 1) * 512)
                        ot = ost_pool.tile([128, 512], F32, tag="ot", bufs=4, name=f"ot_{e}_{mt}_{n}")
                        nc.vector.tensor_copy(ot[:], po[n][:])
                        nc.sync.dma_start(out_d[rows, ncol], ot[:])
    nc.compile()
    return nc


def _in_map_for_core(x, w13, w2, c):
    xs = x[c * ROWS:(c + 1) * ROWS]                      # [4096, 2048] f32
    xt = xs.T.astype(NPBF16, order="C").reshape(KH, 128, ROWS)
    w13c = np.ascontiguousarray(w13[c * EPC:(c + 1) * EPC]).astype(NPBF16)
    w2c = np.ascontiguousarray(w2[c * EPC:(c + 1) * EPC]).astype(NPBF16)
    return {
        "xt": xt,
        "w13": w13c.reshape(EPC, KH, 128, 2 * I),
        "w2": w2c.reshape(EPC, KI, 128, H),
    }


def kernel(x, w13, w2, tokens_per_expert, decoding, _trace=False):
    x = np.asarray(x, dtype=np.float32)
    w13 = np.asarray(w13, dtype=np.float32)
    w2 = np.asarray(w2, dtype=np.float32)

    in_maps = [_in_map_for_core(x, w13, w2, c) for c in range(N_CORES)]
    nc = build_nc()
    res = run_bass_kernel_spmd(nc, in_maps, list(range(N_CORES)), trace=_trace)
    out = np.concatenate([res.results[c]["out"] for c in range(N_CORES)], axis=0)
    if _trace:
        return out, res
    return out



# revision 4
# speedup vs baseline: 74.6074x; 74.6074x over previous
"""MoE grouped-GEMM expert FFN (SwiGLU) on 8 Trainium2 NeuronCores.

Expert-parallel sharding: tokens arrive pre-grouped by expert with uniform
group size g = T/E = 1024, so core c owns experts [4c, 4c+4) and token rows
[c*4096, (c+1)*4096). No cross-core communication is needed: each core
computes its own 4 experts' FFN on its own token block.

Per-core math, per expert e:
    gu^T = w13_e^T-chunks @ x_e^T        # PE: contract H on partitions
    h^T  = silu(gate^T) * up^T           # ACT (Silu) + DVE (mul), bf16
    out  = h @ w2_e                      # PE: contract I on partitions

Dataflow: weights are host-packed into use-once streaming tiles (w13 per
(expert, m-tile): [128, 16k, 256]; w2 per (expert, n-chunk): [128, 8k, 512])
so each 1MB pack is DMA'd just-in-time and released right after its k-loop,
keeping the SBUF working set small and the prefetch pipeline deep (bufs=3).
x^T is one 4MB DMA per expert, double-buffered across experts. Phase-1 runs
the gate k-loop (banks pg0/pg1) then the up k-loop (pu0/pu1) so PSUM bank
evacuation always hides behind ~7us of matmul; phase-2 rotates 4 po banks
across quarters of 2 token-tiles. Output is written bf16 (halves the write
traffic) and upcast to f32 on the host.
"""

import sys

if "/opt/trn_rl_repo" not in sys.path:
    sys.path.insert(0, "/opt/trn_rl_repo")

import ml_dtypes
import numpy as np

import concourse.bacc as bacc
import concourse.bass as bass
import concourse.mybir as mybir
from concourse import tile
from concourse.bass_utils import run_bass_kernel_spmd

BF16 = mybir.dt.bfloat16
F32 = mybir.dt.float32
NPBF16 = ml_dtypes.bfloat16

N_CORES = 8
E = 32
H = 2048
I = 1024
T = 32768
EPC = E // N_CORES          # experts per core = 4
G = T // E                  # tokens per expert = 1024
ROWS = EPC * G              # token rows per core = 4096
KH = H // 128               # 16 contraction tiles for GEMM1
KI = I // 128               # 8 contraction tiles for GEMM2
NH = H // 512               # 4 H-chunks for GEMM2 output


def build_nc(reps=1):
    nc = bacc.Bacc()
    xt_d = nc.declare_dram_parameter("xt", [EPC, 128, KH, G], BF16, isOutput=False)
    w13_d = nc.declare_dram_parameter("w13", [EPC, KI, 128, KH, 256], BF16, isOutput=False)
    w2_d = nc.declare_dram_parameter("w2", [EPC, NH, 128, KI, 512], BF16, isOutput=False)
    out_d = nc.declare_dram_parameter("out", [ROWS, H], BF16, isOutput=True)

    with tile.TileContext(nc) as tc:
        with (
            tc.tile_pool(name="xt", bufs=2) as xt_pool,
            tc.tile_pool(name="w13", bufs=3) as w13_pool,
            tc.tile_pool(name="w2", bufs=3) as w2_pool,
            tc.tile_pool(name="h", bufs=2) as h_pool,
            tc.tile_pool(name="tmp", bufs=3) as tmp_pool,
            tc.tile_pool(name="ost", bufs=4) as ost_pool,
            tc.tile_pool(name="ps", bufs=1, space="PSUM") as ps_pool,
        ):
            for it in range(EPC * reps):
                e = it % EPC
                xt = xt_pool.tile([128, KH, G], BF16, tag="xt", bufs=2, name=f"xt_{it}")
                nc.sync.dma_start(xt[:], xt_d[e][:])

                # Phase 1: gu^T tiles -> SwiGLU -> h^T resident in SBUF (bf16).
                h_sb = [h_pool.tile([128, G], BF16, tag=f"h{m}", bufs=2, name=f"h{m}_{it}")
                        for m in range(KI)]
                for m in range(KI):
                    w13m = w13_pool.tile([128, KH, 256], BF16, tag="w13", bufs=3,
                                         name=f"w13_{it}_{m}")
                    nc.sync.dma_start(w13m[:], w13_d[e, m][:])
                    pg = [ps_pool.tile([128, 512], F32, tag=f"pg{n}", bufs=1,
                                       name=f"pg{n}_{it}_{m}") for n in range(2)]
                    pu = [ps_pool.tile([128, 512], F32, tag=f"pu{n}", bufs=1,
                                       name=f"pu{n}_{it}_{m}") for n in range(2)]
                    # gate k-loop first, then up k-loop: the gate banks are
                    # evacuated by ACT while the up k-loop (~7us) runs, so the
                    # next m-group's first matmul never waits on a bank.
                    for k in range(KH):
                        wg = w13m[:, k, 0:128]
                        for n in range(2):
                            nc.tensor.matmul(
                                pg[n][:], wg, xt[:, k, n * 512:(n + 1) * 512],
                                start=(k == 0), stop=(k == KH - 1),
                            )
                    for k in range(KH):
                        wu = w13m[:, k, 128:256]
                        for n in range(2):
                            nc.tensor.matmul(
                                pu[n][:], wu, xt[:, k, n * 512:(n + 1) * 512],
                                start=(k == 0), stop=(k == KH - 1),
                            )
                    for n in range(2):
                        ncol = slice(n * 512, (n + 1) * 512)
                        tmp = tmp_pool.tile([128, 512], BF16, tag="tmp", bufs=3,
                                            name=f"tmp_{it}_{m}_{n}")
                        pu_sb = tmp_pool.tile([128, 512], BF16, tag="pusb", bufs=3,
                                              name=f"pusb_{it}_{m}_{n}")
                        nc.scalar.activation(
                            tmp[:], pg[n][:], mybir.ActivationFunctionType.Silu
                        )
                        # Both epilogue producers run on ACT so the DVE mul
                        # carries ONE merged ACT wait (the TT instruction
                        # encoding only fits a single sync-wait).
                        nc.scalar.copy(pu_sb[:], pu[n][:])
                        nc.vector.tensor_mul(h_sb[m][:, ncol], tmp[:], pu_sb[:])

                # Phase 2: out_e = h @ w2_e, streamed straight to DRAM.
                for nn in range(NH):
                    w2n = w2_pool.tile([128, KI, 512], BF16, tag="w2", bufs=3,
                                       name=f"w2_{it}_{nn}")
                    nc.sync.dma_start(w2n[:], w2_d[e, nn][:])
                    for q in range(4):      # quarters of 2 token-tiles
                        po = [ps_pool.tile([128, 512], F32, tag=f"po{(2 * q + j) % 4}",
                                           bufs=1, name=f"po{j}_{it}_{nn}_{q}")
                              for j in range(2)]
                        for k in range(KI):
                            for j in range(2):
                                mt = q * 2 + j
                                nc.tensor.matmul(
                                    po[j][:], h_sb[k][:, mt * 128:(mt + 1) * 128],
                                    w2n[:, k, :],
                                    start=(k == 0), stop=(k == KI - 1),
                                )
                        for j in range(2):
                            mt = q * 2 + j
                            rows = slice(e * G + mt * 128, e * G + (mt + 1) * 128)
                            ncol = slice(nn * 512, (nn + 1) * 512)
                            ot = ost_pool.tile([128, 512], BF16, tag="ot", bufs=4,
                                               name=f"ot_{it}_{nn}_{q}_{j}")
                            nc.vector.tensor_copy(ot[:], po[j][:])
                            nc.sync.dma_start(out_d[rows, ncol], ot[:])
    nc.compile()
    return nc


def _in_map_for_core(x, w13, w2, c):
    xs = x[c * ROWS:(c + 1) * ROWS]                      # [4096, 2048] f32
    # xt[e, p, k, g] = x[e*G + g, k*128 + p]
    xt = (xs.reshape(EPC, G, KH, 128).transpose(0, 3, 2, 1)
          .astype(NPBF16, order="C"))
    w13c = w13[c * EPC:(c + 1) * EPC]                    # [EPC, 2048, 2048]
    w13r = w13c.reshape(EPC, KH, 128, 2 * I)
    gate = w13r[:, :, :, :I].reshape(EPC, KH, 128, KI, 128)
    up = w13r[:, :, :, I:].reshape(EPC, KH, 128, KI, 128)
    # w13p[e, m, p, k, 0:128]=gate[e,k,p,m,:], [...,128:256]=up[e,k,p,m,:]
    w13p = np.concatenate([gate, up], axis=-1)           # [EPC, KH, 128, KI, 256]
    w13p = w13p.transpose(0, 3, 2, 1, 4).astype(NPBF16, order="C")
    w2c = w2[c * EPC:(c + 1) * EPC]                      # [EPC, 1024, 2048]
    w2r = w2c.reshape(EPC, KI, 128, NH, 512)
    # w2p[e, n, p, k, :] = w2[e, k*128+p, n*512:(n+1)*512]
    w2p = w2r.transpose(0, 3, 2, 1, 4).astype(NPBF16, order="C")
    return {"xt": xt, "w13": w13p, "w2": w2p}


_NC_CACHE = None


def kernel(x, w13, w2, tokens_per_expert, decoding, _trace=False):
    global _NC_CACHE
    x = np.asarray(x, dtype=np.float32)
    w13 = np.asarray(w13, dtype=np.float32)
    w2 = np.asarray(w2, dtype=np.float32)

    in_maps = [_in_map_for_core(x, w13, w2, c) for c in range(N_CORES)]
    if _NC_CACHE is None:
        _NC_CACHE = build_nc()
    nc = _NC_CACHE
    res = run_bass_kernel_spmd(nc, in_maps, list(range(N_CORES)), trace=_trace)
    out = np.concatenate(
        [res.results[c]["out"].astype(np.float32) for c in range(N_CORES)], axis=0
    )
    if _trace:
        return out, res
    return out


# revision 6
# speedup vs baseline: 83.0066x; 1.1126x over previous
"""MoE grouped-GEMM expert FFN (SwiGLU) on 8 Trainium2 NeuronCores.

Expert-parallel sharding: tokens arrive pre-grouped by expert with uniform
group size g = T/E = 1024, so core c owns experts [4c, 4c+4) and token rows
[c*4096, (c+1)*4096). No cross-core communication is needed: each core
computes its own 4 experts' FFN on its own token block.

Per-core math, per expert e:
    gu^T = w13_e^T-chunks @ x_e^T        # PE: contract H on partitions
    h^T  = silu(gate^T) * up^T           # ACT (Silu) + DVE (mul), bf16
    out  = h @ w2_e                      # PE: contract I on partitions

Dataflow: weights are host-packed into use-once streaming tiles (w13 per
(expert, m-tile): [128, 16k, 256]; w2 per (expert, n-chunk): [128, 8k, 512])
so each 1MB pack is DMA'd just-in-time and released right after its k-loop,
keeping the SBUF working set small and the prefetch pipeline deep (bufs=3).
x^T is one 4MB DMA per expert, double-buffered across experts. Phase-1 runs
the gate k-loop (banks pg0/pg1) then the up k-loop (pu0/pu1) so PSUM bank
evacuation always hides behind ~7us of matmul; phase-2 rotates 4 po banks
across quarters of 2 token-tiles. Output is written bf16 (halves the write
traffic) and upcast to f32 on the host.
"""

import sys

if "/opt/trn_rl_repo" not in sys.path:
    sys.path.insert(0, "/opt/trn_rl_repo")

import ml_dtypes
import numpy as np

import concourse.bacc as bacc
import concourse.bass as bass
import concourse.mybir as mybir
from concourse import tile
from concourse.bass_utils import run_bass_kernel_spmd

BF16 = mybir.dt.bfloat16
F32 = mybir.dt.float32
NPBF16 = ml_dtypes.bfloat16

N_CORES = 8
E = 32
H = 2048
I = 1024
T = 32768
EPC = E // N_CORES          # experts per core = 4
G = T // E                  # tokens per expert = 1024
ROWS = EPC * G              # token rows per core = 4096
KH = H // 128               # 16 contraction tiles for GEMM1
KI = I // 128               # 8 contraction tiles for GEMM2
NH = H // 512               # 4 H-chunks for GEMM2 output


def build_nc(reps=1):
    nc = bacc.Bacc()
    xt_d = nc.declare_dram_parameter("xt", [EPC, 128, KH, G], BF16, isOutput=False)
    w13_d = nc.declare_dram_parameter("w13", [EPC, KI, 128, KH, 256], BF16, isOutput=False)
    w2_d = nc.declare_dram_parameter("w2", [EPC, 128, KI, H], BF16, isOutput=False)
    out_d = nc.declare_dram_parameter("out", [ROWS, H], BF16, isOutput=True)

    with tile.TileContext(nc) as tc:
        with (
            tc.tile_pool(name="xt", bufs=2) as xt_pool,
            tc.tile_pool(name="w13", bufs=3) as w13_pool,
            tc.tile_pool(name="w2", bufs=2) as w2_pool,
            tc.tile_pool(name="h", bufs=2) as h_pool,
            tc.tile_pool(name="tmp", bufs=3) as tmp_pool,
            tc.tile_pool(name="ost", bufs=4) as ost_pool,
            tc.tile_pool(name="ps", bufs=1, space="PSUM") as ps_pool,
        ):
            for it in range(EPC * reps):
                e = it % EPC
                xt = xt_pool.tile([128, KH, G], BF16, tag="xt", bufs=2, name=f"xt_{it}")
                nc.sync.dma_start(xt[:], xt_d[e][:])
                w2e = w2_pool.tile([128, KI, H], BF16, tag="w2", bufs=2, name=f"w2_{it}")
                nc.sync.dma_start(w2e[:], w2_d[e][:])

                # Phase 1: gu^T tiles -> SwiGLU -> h^T resident in SBUF (bf16).
                h_sb = [h_pool.tile([128, G], BF16, tag=f"h{m}", bufs=2, name=f"h{m}_{it}")
                        for m in range(KI)]
                for m in range(KI):
                    w13m = w13_pool.tile([128, KH, 256], BF16, tag="w13", bufs=3,
                                         name=f"w13_{it}_{m}")
                    nc.sync.dma_start(w13m[:], w13_d[e, m][:])
                    pg = [ps_pool.tile([128, 512], F32, tag=f"pg{n}", bufs=1,
                                       name=f"pg{n}_{it}_{m}") for n in range(2)]
                    pu = [ps_pool.tile([128, 512], F32, tag=f"pu{n}", bufs=1,
                                       name=f"pu{n}_{it}_{m}") for n in range(2)]
                    # Interleave gate/up inside the k loop: each stationary
                    # feeds two consecutive matmuls (LDW reuse) and the PE
                    # pipeline fills across the four banks.
                    for k in range(KH):
                        wg = w13m[:, k, 0:128]
                        wu = w13m[:, k, 128:256]
                        for n in range(2):
                            nc.tensor.matmul(
                                pg[n][:], wg, xt[:, k, n * 512:(n + 1) * 512],
                                start=(k == 0), stop=(k == KH - 1),
                            )
                        for n in range(2):
                            nc.tensor.matmul(
                                pu[n][:], wu, xt[:, k, n * 512:(n + 1) * 512],
                                start=(k == 0), stop=(k == KH - 1),
                            )
                    for n in range(2):
                        ncol = slice(n * 512, (n + 1) * 512)
                        tmp = tmp_pool.tile([128, 512], BF16, tag="tmp", bufs=3,
                                            name=f"tmp_{it}_{m}_{n}")
                        pu_sb = tmp_pool.tile([128, 512], BF16, tag="pusb", bufs=3,
                                              name=f"pusb_{it}_{m}_{n}")
                        nc.scalar.activation(
                            tmp[:], pg[n][:], mybir.ActivationFunctionType.Silu
                        )
                        # Both epilogue producers run on ACT so the DVE mul
                        # carries ONE merged ACT wait (the TT instruction
                        # encoding only fits a single sync-wait).
                        nc.scalar.copy(pu_sb[:], pu[n][:])
                        nc.vector.tensor_mul(h_sb[m][:, ncol], tmp[:], pu_sb[:])

                # Phase 2: out_e = h @ w2_e, streamed straight to DRAM.
                # One stationary h-tile feeds 4 matmuls (the 4 H-chunks).
                for mt in range(KI):
                    rows = slice(e * G + mt * 128, e * G + (mt + 1) * 128)
                    po = [ps_pool.tile([128, 512], F32, tag=f"po{n}", bufs=1,
                                       name=f"po{n}_{it}_{mt}") for n in range(4)]
                    for k in range(KI):
                        hk = h_sb[k][:, mt * 128:(mt + 1) * 128]
                        for n in range(4):
                            nc.tensor.matmul(
                                po[n][:], hk, w2e[:, k, n * 512:(n + 1) * 512],
                                start=(k == 0), stop=(k == KI - 1),
                            )
                    for n in range(4):
                        ncol = slice(n * 512, (n + 1) * 512)
                        ot = ost_pool.tile([128, 512], BF16, tag="ot", bufs=4,
                                           name=f"ot_{it}_{mt}_{n}")
                        nc.vector.tensor_copy(ot[:], po[n][:])
                        nc.sync.dma_start(out_d[rows, ncol], ot[:])
    nc.compile()
    return nc


def _in_map_for_core(x, w13, w2, c):
    xs = x[c * ROWS:(c + 1) * ROWS]                      # [4096, 2048] f32
    # xt[e, p, k, g] = x[e*G + g, k*128 + p]
    xt = (xs.reshape(EPC, G, KH, 128).transpose(0, 3, 2, 1)
          .astype(NPBF16, order="C"))
    w13c = w13[c * EPC:(c + 1) * EPC]                    # [EPC, 2048, 2048]
    w13r = w13c.reshape(EPC, KH, 128, 2 * I)
    gate = w13r[:, :, :, :I].reshape(EPC, KH, 128, KI, 128)
    up = w13r[:, :, :, I:].reshape(EPC, KH, 128, KI, 128)
    # w13p[e, m, p, k, 0:128]=gate[e,k,p,m,:], [...,128:256]=up[e,k,p,m,:]
    w13p = np.concatenate([gate, up], axis=-1)           # [EPC, KH, 128, KI, 256]
    w13p = w13p.transpose(0, 3, 2, 1, 4).astype(NPBF16, order="C")
    w2c = w2[c * EPC:(c + 1) * EPC]                      # [EPC, 1024, 2048]
    w2r = w2c.reshape(EPC, KI, 128, H)
    # w2p[e, p, k, :] = w2[e, k*128+p, :]
    w2p = w2r.transpose(0, 2, 1, 3).astype(NPBF16, order="C")
    return {"xt": xt, "w13": w13p, "w2": w2p}


_NC_CACHE = None


def kernel(x, w13, w2, tokens_per_expert, decoding, _trace=False):
    global _NC_CACHE
    x = np.asarray(x, dtype=np.float32)
    w13 = np.asarray(w13, dtype=np.float32)
    w2 = np.asarray(w2, dtype=np.float32)

    in_maps = [_in_map_for_core(x, w13, w2, c) for c in range(N_CORES)]
    if _NC_CACHE is None:
        _NC_CACHE = build_nc()
    nc = _NC_CACHE
    res = run_bass_kernel_spmd(nc, in_maps, list(range(N_CORES)), trace=_trace)
    out = np.concatenate(
        [res.results[c]["out"].astype(np.float32) for c in range(N_CORES)], axis=0
    )
    if _trace:
        return out, res
    return out
